# revision 20
# baseline (speedup 1.0000x reference)
"""Trainium2 Bass kernel: causal multi-head self-attention (b=2, s=2048, d=1024, h=16).

Distribution (8 NeuronCores, SPMD single program):
  - Tensor-parallel over heads: core c owns heads {c, c+8} (hl=0 -> head c,
    hl=1 -> head c+8). It computes those heads' Q/K/V projections over the
    full sequence, then causal attention for its heads.
  - AllToAll (one per head-half) redistributes the attention output from
    head-sharded [all rows, 64 cols] to row-sharded [512 rows, all 1024 cols].
    With the {c, c+8} assignment, a2a(0) delivers exactly Wo-row blocks 0-3
    and a2a(1) blocks 4-7, so half of the output projection (contraction
    blocks dc 0-3) runs as soon as the first all-to-all lands, hiding it
    under the second one.
  - Output projection is row-parallel (full Wo on every core); host
    concatenates the per-core row blocks.

Precision: projections use residual-split fp8 DoubleRow matmuls
(x = x8 + xr8, W = w8 + wr8, all fp8e4m3; q = x8@w8 + x8@wr8 + xr8@w8, the
dropped xr8@wr8 term is ~0.13% of one summand). PSUM accumulates fp32; the
stored q/k/v and all attention arithmetic are bf16 exactly as the all-bf16
variant, so the end-to-end error matches bf16 while the projection matmuls
run at fp8 DoubleRow rate. Scores/PV/out-proj stay bf16 (fp8 there fails the
2e-2 gate). Softmax needs no max-subtraction (scores are O(5)); the
denominator is a 65th "ones" column appended to V.
"""

import sys

for _p in ("/opt/trn_rl_repo",):
    if _p not in sys.path:
        sys.path.insert(0, _p)

import numpy as np
import ml_dtypes

import concourse.bass as bass
import concourse.mybir as mybir
import concourse.tile as tile
from concourse import bacc
from concourse.bass_utils import run_bass_kernel_spmd

BF16 = mybir.dt.bfloat16
F8 = mybir.dt.float8e4
F32 = mybir.dt.float32
AF = mybir.ActivationFunctionType
DR = mybir.MatmulPerfMode.DoubleRow

B, S, D, H, DK = 2, 2048, 1024, 16, 64
NROWS = B * S          # 4096 flattened (batch, seq) rows
NC = 8                 # cores
HPC = H // NC          # 2 heads per core
DHC = HPC * DK         # 128 head-dim columns per core
RPC = NROWS // NC      # 512 output rows per core
QB = 16                # 128-row query blocks per batch
SCALE = 1.0 / float(np.sqrt(DK))
EXP_SCALE = SCALE / 1024.0  # q,k stored at 32x (fp8 weight scale)


def _build_kernel(nc: bass.Bass, single_core: bool = False):
    # x and the three projection weights are residual-split fp8 pairs:
    # index 0 = fp8(x), index 1 = fp8(x - fp8(x)).
    x2 = nc.dram_tensor("x2", [128, 2, 8, NROWS], F8, kind="ExternalInput")
    wq2 = nc.dram_tensor("wq2", [128, 2, 8, DHC], F8, kind="ExternalInput")
    wk2 = nc.dram_tensor("wk2", [128, 2, 8, DHC], F8, kind="ExternalInput")
    wv2 = nc.dram_tensor("wv2", [128, 2, 8, DHC], F8, kind="ExternalInput")
    wo = nc.dram_tensor("wo", [D, D], BF16, kind="ExternalInput")
    maskin = nc.dram_tensor("maskin", [128, 128], BF16, kind="ExternalInput")
    identin = nc.dram_tensor("identin", [128, 128], BF16, kind="ExternalInput")
    out = nc.dram_tensor("out", [RPC, D], F32, kind="ExternalOutput")

    with tile.TileContext(nc) as tc:
        _body(tc, x2, wq2, wk2, wv2, wo, maskin, identin, out, single_core)


def _body(tc, x2, wq2, wk2, wv2, wo, maskin, identin, out, single_core=False):
    nc = tc.nc
    from contextlib import ExitStack

    with ExitStack() as ctx:
        const_pool = ctx.enter_context(tc.tile_pool(name="const", bufs=1))
        proj_pool = ctx.enter_context(tc.tile_pool(name="proj", bufs=1))
        x_pool = ctx.enter_context(tc.tile_pool(name="x", bufs=3))
        w_pool = ctx.enter_context(tc.tile_pool(name="w", bufs=1))
        # PSUM budget (8 banks): mm512 2 + st 4 + acc 2
        psum_pool = ctx.enter_context(
            tc.tile_pool(name="psum", bufs=2, space="PSUM")
        )
        sb_pool = ctx.enter_context(tc.tile_pool(name="sb", bufs=4))
        part_pool = ctx.enter_context(tc.tile_pool(name="part", bufs=1))
        dram_pool = ctx.enter_context(
            tc.tile_pool(name="dram", bufs=1, space="DRAM")
        )

        # ---- weights + constants ----------------------------------------
        # [2, D, M] -> sbuf [128, 2, D//128, M] (partition = din % 128).
        wq_sb = w_pool.tile([128, 2, 8, DHC], F8, tag="wq")
        wk_sb = w_pool.tile([128, 2, 8, DHC], F8, tag="wk")
        wv_sb = w_pool.tile([128, 2, 8, DHC], F8, tag="wv")
        wo_sb = w_pool.tile([128, 8, D], BF16, tag="wo")
        nc.sync.dma_start(wq_sb[:, :, 0:4, :], wq2[:, :, 0:4, :])
        nc.sync.dma_start(wq_sb[:, :, 4:8, :], wq2[:, :, 4:8, :])
        mask_sb = const_pool.tile([128, 128], BF16)
        ident_sb = const_pool.tile([128, 128], BF16)

        # ---- projections: qT/kT/vT [128 (hl*64+dk), 4096] bf16 ----------
        qT = proj_pool.tile([128, NROWS], BF16, tag="qT")
        kT = proj_pool.tile([128, NROWS], BF16, tag="kT")
        vT = proj_pool.tile([128, NROWS], BF16, tag="vT")
        v_aug = proj_pool.tile([128, 4, QB, DK + 1], BF16, tag="vaug")
        xT_r = x2.ap()

        def build_vaug(b, c0s):
            # Transpose 128-row chunks of vT (both heads at once) into the
            # natural [k, dk] layout; 65th column = ones (softmax denom).
            for hl in range(2):
                if 0 in c0s:
                    # v is stored at 32x (fp8 weight scale); a 32.0 "ones"
                    # column makes the denominator scale match exactly.
                    nc.vector.memset(
                        v_aug[:, hl * 2 + b, :, DK : DK + 1], 32.0
                    )
            for c0 in c0s:
                pt = psum_pool.tile([128, 8, 128], BF16, tag="mm512")
                for ci in range(8):
                    col0 = b * S + (c0 + ci) * 128
                    nc.tensor.transpose(
                        pt[:, ci, :],
                        vT[:, col0 : col0 + 128],
                        ident_sb[:, :],
                    )
                for hl in range(2):
                    nc.vector.tensor_copy(
                        v_aug[:, hl * 2 + b, c0 : c0 + 8, 0:DK],
                        pt[:, :, hl * DK : hl * DK + DK],
                    )

        def proj_group(g):
            xg = x_pool.tile([128, 2, 8, 512], F8, tag="xg")
            if g == 0:
                nc.sync.dma_start(
                    xg[:, 0, 0:4, :], xT_r[:, 0, 0:4, 0:512]
                )
                nc.sync.dma_start(
                    xg[:, 0, 4:8, :], xT_r[:, 0, 4:8, 0:512]
                )
                nc.sync.dma_start(
                    xg[:, 1, :, :], xT_r[:, 1, :, 0:512]
                )
                nc.sync.dma_start(wk_sb[:], wk2[:, :, :, :])
                nc.sync.dma_start(wv_sb[:], wv2[:, :, :, :])
                nc.sync.dma_start(ident_sb[:], identin[:, :])
                nc.sync.dma_start(mask_sb[:], maskin[:, :])
            else:
                nc.sync.dma_start(
                    xg[:], xT_r[:, :, :, g * 512 : (g + 1) * 512]
                )
            for w_sb, projT, eng in (
                (wq_sb, qT, nc.vector),
                (wk_sb, kT, nc.vector),
                (wv_sb, vT, nc.vector),
            ):
                ps = psum_pool.tile([128, 512], F32, tag="mm512")
                # residual-split fp8: x8@w8 + x8@wr8 + xr8@w8, each as
                # DoubleRow chains over din-chunk pairs. Order the matmuls
                # so each only needs DMA pieces that have already landed.
                if g == 0:
                    order = [(0, 0, 0), (0, 1, 0), (1, 0, 0), (1, 1, 0),
                             (0, 0, 2), (0, 1, 2), (1, 0, 2), (1, 1, 2),
                             (0, 0, 1), (0, 1, 1), (1, 0, 1), (1, 1, 1)]
                    # (dp_half, dp_in_half, term): encoded below
                    seq = [(0, 0), (0, 1), (1, 0), (1, 1),
                           (2, 0), (2, 1), (3, 0), (3, 1),
                           (0, 2), (1, 2), (2, 2), (3, 2)]
                else:
                    seq = [(dp, ti) for ti in range(3) for dp in range(4)]
                terms = ((0, 0), (0, 1), (1, 0))  # x8w8, x8wr8, xr8w8
                for i, (dp, ti) in enumerate(seq):
                    xs, ws = terms[ti]
                    nc.tensor.matmul(
                        ps[:],
                        w_sb[:, ws, 2 * dp : 2 * dp + 2, :],
                        xg[:, xs, 2 * dp : 2 * dp + 2, :],
                        start=(i == 0),
                        stop=(i == 11),
                        perf_mode=DR,
                    )
                eng.tensor_copy(projT[:, g * 512 : (g + 1) * 512], ps[:])

        # ---- attention -------------------------------------------------
        send_h = [
            dram_pool.tile(
                [NC, DK, RPC], BF16, tag=f"send{hl}", name=f"send_h{hl}"
            )
            for hl in range(2)
        ]
        recv_h = [
            dram_pool.tile(
                [NC, DK, RPC], BF16, tag=f"recv{hl}", name=f"recv_h{hl}"
            )
            for hl in range(2)
        ]

        def attend_group(hl, b, g, pe_bcast=False):
            pair = hl * 2 + b
            hs = hl * DK
            qcol0 = b * S + g * 512
            nck = 4 * g + 4
            acc = psum_pool.tile([DK + 1, 512], F32, tag="acc")
            for ci in range(0, nck, 2):
                st = psum_pool.tile([128, 2, 512], F32, tag="st")
                if ci + 2 <= 4 * g:
                    # below the diagonal band: full-width, batched exp
                    for j in range(2):
                        kcol0 = b * S + (ci + j) * 128
                        nc.tensor.matmul(
                            st[:, j, :],
                            kT[hs : hs + DK, kcol0 : kcol0 + 128],
                            qT[hs : hs + DK, qcol0 : qcol0 + 512],
                            start=True,
                            stop=True,
                        )
                    p_t = sb_pool.tile([128, 2, 512], BF16, tag="pt")
                    nc.scalar.activation(
                        p_t[:, :, :], st[:, :, :], AF.Exp, scale=EXP_SCALE
                    )
                    for j in range(2):
                        ck = ci + j
                        nc.tensor.matmul(
                            acc[:],
                            v_aug[:, pair, ck, :],
                            p_t[:, j, :],
                            start=(ck == 0),
                            stop=False,
                        )
                else:
                    # diagonal band: only q columns >= r*128 are live.
                    p_t = sb_pool.tile([128, 2, 512], BF16, tag="pt")
                    r0 = ci - 4 * g
                    c0u = r0 * 128
                    for j in range(2):
                        ck = ci + j
                        kcol0 = b * S + ck * 128
                        nc.tensor.matmul(
                            st[:, j, c0u:512],
                            kT[hs : hs + DK, kcol0 : kcol0 + 128],
                            qT[hs : hs + DK, qcol0 + c0u : qcol0 + 512],
                            start=True,
                            stop=True,
                        )
                    nc.scalar.activation(
                        p_t[:, :, c0u:512],
                        st[:, :, c0u:512],
                        AF.Exp,
                        scale=EXP_SCALE,
                    )
                    for j in range(2):
                        ck = ci + j
                        r = ck - 4 * g
                        c0 = r * 128
                        nc.vector.tensor_mul(
                            p_t[:, j, c0 : c0 + 128],
                            p_t[:, j, c0 : c0 + 128],
                            mask_sb[:],
                        )
                        nc.tensor.matmul(
                            acc[:, c0:512],
                            v_aug[:, pair, ck, :],
                            p_t[:, j, c0:512],
                            start=(ck == 0),
                            stop=(ck == nck - 1),
                        )
            slab = sb_pool.tile([DK, 512], BF16, tag="slab")
            if pe_bcast:
                # tail-critical group: pipeline the normalization in two
                # q-halves so the send DMA fires sooner
                for h0 in (0, 256):
                    recip = sb_pool.tile([1, 256], F32, tag="recipb")
                    nc.vector.reciprocal(
                        recip[:], acc[DK : DK + 1, h0 : h0 + 256]
                    )
                    bcast = sb_pool.tile([DK, 256], F32, tag="bcastb")
                    nc.gpsimd.partition_broadcast(bcast[:], recip[:])
                    nc.vector.tensor_mul(
                        slab[:, h0 : h0 + 256],
                        acc[0:DK, h0 : h0 + 256],
                        bcast[:],
                    )
            else:
                recip = sb_pool.tile([1, 512], F32, tag="recip")
                nc.vector.reciprocal(recip[:], acc[DK : DK + 1, :])
                bcast = sb_pool.tile([DK, 512], F32, tag="bcast")
                nc.gpsimd.partition_broadcast(bcast[:], recip[:])
                nc.vector.tensor_mul(slab[:], acc[0:DK, :], bcast[:])
            dest = b * 4 + g
            nc.sync.dma_start(send_h[hl][dest, :, :], slab[:])

        def a2a(hl):
            if single_core:
                nc.sync.dma_start(recv_h[hl][:], send_h[hl][:])
            else:
                nc.gpsimd.collective_compute(
                    "AllToAll",
                    mybir.AluOpType.bypass,
                    replica_groups=[list(range(NC))],
                    ins=[send_h[hl].opt()],
                    outs=[recv_h[hl].opt()],
                )

        # head h < 8 lives on core h (hl=0); head h >= 8 on core h-8 (hl=1).
        # Wo row block dc needs heads {2dc, 2dc+1}: dc 0-3 come entirely from
        # a2a(0), dc 4-7 from a2a(1).
        attnT = proj_pool.tile([128, 8, RPC], BF16, tag="attnT")

        def gather(hl):
            # attnT[two*64+k, dc, q] = recv_h[hl][2*(dc-4*hl)+two, k, q]
            rr = recv_h[hl][:, :, :].rearrange(
                "(s2 two) k q -> (two k) s2 q", two=2
            )
            for j in range(2):
                nc.scalar.dma_start(
                    attnT[:, 4 * hl + 2 * j : 4 * hl + 2 * j + 2, :],
                    rr[:, 2 * j : 2 * j + 2, :],
                )

        # 8 persistent accumulators [128, 512] carved out of the freed
        # attention psum rings (attention has drained by the time these run)
        po_tiles = {}

        def outproj_a(qb, half):
            # contraction blocks dc 0-3 (after a2a(0)); runs in the a2a(1)
            # window; accumulation stays open until phase B
            po = po_tiles[qb * 2 + half]
            for dc in range(4):
                nc.tensor.matmul(
                    po,
                    attnT[:, dc, qb * 128 : (qb + 1) * 128],
                    wo_sb[:, dc, half * 512 : (half + 1) * 512],
                    start=(dc == 0),
                    stop=False,
                )

        def outproj_b(qb):
            orow = sb_pool.tile([128, D], F32, tag="orow")
            for half in range(2):
                po = po_tiles[qb * 2 + half]
                for dc in range(4, 8):
                    nc.tensor.matmul(
                        po,
                        attnT[:, dc, qb * 128 : (qb + 1) * 128],
                        wo_sb[:, dc, half * 512 : (half + 1) * 512],
                        start=False,
                        stop=(dc == 7),
                    )
                eng = nc.vector if half == 0 else nc.scalar
                if half == 0:
                    nc.vector.tensor_copy(
                        orow[:, 0:512], po
                    )
                else:
                    nc.scalar.activation(
                        orow[:, 512:1024], po, AF.Copy
                    )
                nc.sync.dma_start(
                    out[qb * 128 : (qb + 1) * 128, half * 512 : (half + 1) * 512],
                    orow[:, half * 512 : (half + 1) * 512],
                )

        # ---- schedule ---------------------------------------------------
        # Projections woven with head-0 attention; all of head 0 (both
        # batches) finishes first so a2a(0) fires early, then head-1
        # attention + the dc 0-3 half of the out projection overlap
        # a2a(0)/gather(0); only a2a(1) + the dc 4-7 half are exposed.
        proj_group(0)
        proj_group(1)
        build_vaug(0, [0])
        proj_group(2)
        attend_group(0, 0, 0)
        proj_group(3)
        attend_group(0, 0, 1)
        build_vaug(0, [8])
        proj_group(4)
        attend_group(0, 0, 2)
        proj_group(5)
        attend_group(0, 0, 3)
        proj_group(6)
        build_vaug(1, [0])
        attend_group(0, 1, 0)
        proj_group(7)
        attend_group(1, 0, 0)
        attend_group(0, 1, 1)
        build_vaug(1, [8])
        nc.sync.dma_start(
            wo_sb[:], wo.ap().rearrange("(c p) m -> p c m", p=128)
        )
        attend_group(1, 0, 1)
        attend_group(0, 1, 2)
        attend_group(1, 0, 2)
        attend_group(0, 1, 3)
        a2a(0)
        attend_group(1, 0, 3)
        attend_group(1, 1, 0)
        gather(0)
        attend_group(1, 1, 1)
        attend_group(1, 1, 2)
        attend_group(1, 1, 3)
        for i in range(2):
            t = psum_pool.tile([128, 2, 512], F32, tag="st", name=f"po_st{i}")
            po_tiles[2 * i] = t[:, 0, :]
            po_tiles[2 * i + 1] = t[:, 1, :]
        for i in range(2):
            tm = psum_pool.tile([128, 512], F32, tag="mm512", name=f"po_mm{i}")
            po_tiles[4 + i] = tm[:]
            ta = psum_pool.tile([128, 512], F32, tag="acc", name=f"po_acc{i}")
            po_tiles[6 + i] = ta[:]
        a2a(1)
        outproj_a(0, 0)
        outproj_a(0, 1)
        outproj_a(1, 0)
        outproj_a(1, 1)
        outproj_a(2, 0)
        outproj_a(2, 1)
        outproj_a(3, 0)
        outproj_a(3, 1)
        gather(1)
        for qb in range(4):
            outproj_b(qb)


_CACHE = {}


def _get_compiled():
    if "nc" not in _CACHE:
        nc = bacc.Bacc(
            "TRN2", target_bir_lowering=False, debug=False, num_devices=NC
        )
        _build_kernel(nc)
        nc.compile()
        _CACHE["nc"] = nc
    return _CACHE["nc"]


def _make_in_maps(x, Wq, Wk, Wv, Wo):
    bf = ml_dtypes.bfloat16
    f8 = ml_dtypes.float8_e4m3
    xT = x.reshape(NROWS, D).T.astype(np.float32)
    x8 = xT.astype(f8)
    xr8 = (xT - x8.astype(np.float32)).astype(f8)
    # [2, D, NROWS] -> [128, 2, 8, NROWS] (partition = din % 128)
    x2 = np.ascontiguousarray(
        np.stack([x8, xr8]).reshape(2, 8, 128, NROWS).transpose(2, 0, 1, 3)
    )
    wo = np.ascontiguousarray(Wo.astype(np.float32).astype(bf))
    mask = np.triu(np.ones((128, 128), dtype=np.float32)).astype(bf)
    ident = np.eye(128, dtype=np.float32).astype(bf)

    def wsplit(W, c):
        # columns for heads {c, c+8}
        cols = np.concatenate(
            [
                W[:, c * DK : (c + 1) * DK],
                W[:, (c + 8) * DK : (c + 9) * DK],
            ],
            axis=1,
        ).astype(np.float32) * 32.0
        w8 = cols.astype(f8)
        wr8 = (cols - w8.astype(np.float32)).astype(f8)
        # [2, D, DHC] -> [128, 2, 8, DHC]
        return np.ascontiguousarray(
            np.stack([w8, wr8]).reshape(2, 8, 128, DHC).transpose(2, 0, 1, 3)
        )

    in_maps = []
    for c in range(NC):
        in_maps.append(
            {
                "x2": x2,
                "wq2": wsplit(Wq, c),
                "wk2": wsplit(Wk, c),
                "wv2": wsplit(Wv, c),
                "wo": wo,
                "maskin": mask,
                "identin": ident,
            }
        )
    return in_maps


def _get_runner():
    """Build (once) a cached jitted SPMD executor mirroring
    concourse.bass2jax.run_bass_via_pjrt's multi-core path, so repeat calls
    skip retracing/recompiling the wrapper."""
    if "runner" in _CACHE:
        return _CACHE["runner"]
    import jax
    from jax.sharding import Mesh, PartitionSpec
    from jax.experimental.shard_map import shard_map
    from concourse import bass2jax

    nc = _get_compiled()
    bass2jax.install_neuronx_cc_hook()
    in_names, out_names, out_avals, zero_shapes = [], [], [], []
    partition_name = (
        nc.partition_id_tensor.name if nc.partition_id_tensor else None
    )
    for alloc in nc.m.functions[0].allocations:
        if not isinstance(alloc, mybir.MemoryLocationSet):
            continue
        name = alloc.memorylocations[0].name
        if alloc.kind == "ExternalInput":
            if name != partition_name:
                in_names.append(name)
        elif alloc.kind == "ExternalOutput":
            shape = tuple(alloc.tensor_shape)
            dtype = mybir.dt.np(alloc.dtype)
            out_names.append(name)
            out_avals.append(jax.core.ShapedArray(shape, dtype))
            zero_shapes.append((shape, dtype))
    n_params = len(in_names)
    all_names = in_names + out_names
    if partition_name is not None:
        all_names = all_names + [partition_name]
    all_in_names = tuple(all_names)

    def _body(*args):
        operands = list(args)
        if partition_name is not None:
            operands.append(bass2jax.partition_id_tensor())
        return tuple(
            bass2jax._bass_exec_p.bind(
                *operands,
                out_avals=tuple(out_avals),
                in_names=all_in_names,
                out_names=tuple(out_names),
                lowering_input_output_aliases=(),
                sim_require_finite=True,
                sim_require_nnan=True,
                nc=nc,
            )
        )

    devices = jax.devices()[:NC]
    mesh = Mesh(np.asarray(devices), ("core",))
    nin = n_params + len(out_names)
    sharded = jax.jit(
        shard_map(
            _body,
            mesh=mesh,
            in_specs=(PartitionSpec("core"),) * nin,
            out_specs=(PartitionSpec("core"),) * len(out_names),
            check_rep=False,
        ),
        donate_argnums=tuple(range(n_params, nin)),
        keep_unused=True,
    )

    def run(in_maps):
        concat_in = [
            np.concatenate(
                [np.asarray(in_maps[c][nm]) for c in range(NC)], axis=0
            )
            for nm in in_names
        ]
        concat_zeros = [
            np.zeros((NC * s[0], *s[1:]), dt) for s, dt in zero_shapes
        ]
        out_arrs = sharded(*concat_in, *concat_zeros)
        return [
            {
                name: np.asarray(out_arrs[i]).reshape(
                    NC, *out_avals[i].shape
                )[c]
                for i, name in enumerate(out_names)
            }
            for c in range(NC)
        ]

    _CACHE["runner"] = run
    return run


def kernel(x, Wq, Wk, Wv, Wo, _run_kwargs=None):
    x = np.asarray(x, dtype=np.float32)
    in_maps = _make_in_maps(np.asarray(x), np.asarray(Wq), np.asarray(Wk),
                            np.asarray(Wv), np.asarray(Wo))
    if _run_kwargs:
        nc = _get_compiled()
        res = run_bass_kernel_spmd(
            nc, in_maps, core_ids=list(range(NC)), **_run_kwargs
        )
        _CACHE["last_results"] = res
        results = res.results
    else:
        results = _get_runner()(in_maps)
    outs = [results[c]["out"] for c in range(NC)]
    full = np.concatenate(outs, axis=0)  # [4096, 1024]
    return full.reshape(B, S, D).astype(np.float32)
